# revision 56
# baseline (speedup 1.0000x reference)
"""Trainium2 Bass kernel for MiniCrossAttention (LN -> QK^T -> masked softmax -> AV).

Data-parallel over batch N=8: one batch element per NeuronCore.

Per-core algorithm (T=1024, S=2048, E=512):
  q  = LN(target)         [T,E]   (fp32r tiles, tokens on partitions)
  kv = LN(source)         [S,E+2] (col E = 1.0 -- softmax-denominator trick; col E+1 = 0 pad)
  qT, kvT = PE transposes       ([E,T] / [E,S] layouts, contraction dim on partitions)
  scoresT[s,t] = kvT.T @ qT     (fp32r matmuls, PSUM f32 accum over 4 e-chunks)
  pT = exp(scale*scoresT + maskbias[s])   (ACT, mask folded into per-partition bias)
  out_unnorm[t, 0:E] | denom[t] = pT.T @ kv   (ones-column makes denom a free output col)
  out = out_unnorm * (1/denom)  (DVE reciprocal + tensor_scalar_mul)

Engine split: DVE does bn_stats/bn_aggr + final normalize; ACT does the LN
rstd (exp(-0.5*ln(var+eps)) -- same table set as the softmax Exp, so zero
table switches), all softmax exps, and the PSUM evictions; GPSIMD applies
(x-mean)*rstd; PE does transposes + all matmuls (fp32r = 1 cycle/row).
Input DMAs alternate between the SP and ACT HWDGE queue sets.  kv
transposes, both halves' scoresT+exp, and the first two AV chains all
stream per-j so the PE pipeline never drains; 30 warmup matmuls hold the
PE HAM clock-gate open through the LN head.
"""

import math

import numpy as np
import concourse.bass as bass
import concourse.mybir as mybir
import concourse.tile as tile
from concourse import bacc
from concourse.masks import make_identity
from concourse.bass_utils import run_bass_kernel_spmd

N_CORES = 8
T, S, E = 1024, 2048, 512
P = 128
NT = T // P          # 8 target tiles
NS = S // P          # 16 source tiles
NE = E // P          # 4 e-chunks
EPS = 1e-5
SCALE = 1.0 / float(np.sqrt(E))
MASK_NEG = -30.0     # exp(-30+x) ~ 1e-11: negligible vs denom >= 1

F32 = mybir.dt.float32
F32R = mybir.dt.float32r
AF = mybir.ActivationFunctionType

_cache = {}


class _LnConsts:
    pass


def _emit_ln(nc, io_pool, stats_pool, cst, x_dram, row0, out_tile, dma_eng, affine=None,
             exp_bias=0.0, rstd_out=None):
    """LN one [128, E] tile of x_dram (rows row0:row0+128) into out_tile (fp32r).

    """
    x = io_pool.tile([P, E], F32, tag="ln_x")
    dma_eng.dma_start(out=x[:], in_=x_dram[row0 : row0 + P, :])
    stats = stats_pool.tile([P, nc.vector.BN_STATS_DIM], F32, tag="ln_stats")
    nc.vector.bn_stats(out=stats[:], in_=x[:])
    mv = stats_pool.tile([P, nc.vector.BN_AGGR_DIM], F32, tag="ln_mv")
    nc.vector.bn_aggr(out=mv[:], in_=stats[:])
    # mv[:,0] = mean, mv[:,1] = var -> rstd = exp(-0.5*ln(var+eps)).
    # Ln and Exp share one ACT table set (natural_log_exp), so LN never
    # forces a table switch against the softmax Exp stream.
    nc.scalar.activation(
        out=mv[:, 1:2], in_=mv[:, 1:2], func=AF.Ln, bias=cst.eps[:], scale=1.0
    )
    rdst = mv[:, 1:2] if rstd_out is None else rstd_out
    nc.scalar.activation(
        out=rdst, in_=mv[:, 1:2], func=AF.Exp, bias=exp_bias, scale=-0.5
    )
    if affine is None:
        # out = (x - mean) * rstd on the otherwise-idle GPSIMD engine
        nc.gpsimd.tensor_scalar(
            out=out_tile,
            in0=x[:],
            scalar1=mv[:, 0:1],
            scalar2=rdst,
            op0=mybir.AluOpType.subtract,
            op1=mybir.AluOpType.mult,
        )
        return x
    else:
        w_bcast, b_bcast = affine
        tmp = io_pool.tile([P, E], F32, tag="ln_tmp")
        nc.gpsimd.tensor_scalar(
            out=tmp[:],
            in0=x[:],
            scalar1=mv[:, 0:1],
            scalar2=rdst,
            op0=mybir.AluOpType.subtract,
            op1=mybir.AluOpType.mult,
        )
        nc.vector.tensor_mul(tmp[:], tmp[:], w_bcast[:])
        nc.vector.tensor_add(out_tile, tmp[:], b_bcast[:])
    return x


def _build(apply_affine: bool):
    nc = bacc.Bacc("TRN2", target_bir_lowering=False, debug=False, num_devices=N_CORES)
    target_d = nc.dram_tensor("target_t", [T, E], F32, kind="ExternalInput")
    source_d = nc.dram_tensor("source_t", [S, E], F32, kind="ExternalInput")
    maskb_d = nc.dram_tensor("maskbias", [P, NS], F32, kind="ExternalInput")
    out_d = nc.dram_tensor("out_t", [T, E], F32, kind="ExternalOutput")
    if apply_affine:
        lnw_t_d = nc.dram_tensor("lnw_t", [E], F32, kind="ExternalInput")
        lnb_t_d = nc.dram_tensor("lnb_t", [E], F32, kind="ExternalInput")
        lnw_s_d = nc.dram_tensor("lnw_s", [E], F32, kind="ExternalInput")
        lnb_s_d = nc.dram_tensor("lnb_s", [E], F32, kind="ExternalInput")

    with tile.TileContext(nc) as tc, bass.ExitStack() as ctx:
        const = ctx.enter_context(tc.tile_pool(name="const", bufs=1))
        io_pool = ctx.enter_context(tc.tile_pool(name="io", bufs=6))
        stats_pool = ctx.enter_context(tc.tile_pool(name="stats", bufs=8))
        q_pool = ctx.enter_context(tc.tile_pool(name="q", bufs=1))
        kv_pool = ctx.enter_context(tc.tile_pool(name="kv", bufs=1))
        tr_pool = ctx.enter_context(tc.tile_pool(name="tr", bufs=1))
        p_pool = ctx.enter_context(tc.tile_pool(name="p", bufs=1))
        out_pool = ctx.enter_context(tc.tile_pool(name="o", bufs=3))
        ps_tr = ctx.enter_context(tc.tile_pool(name="ps_tr", bufs=2, space="PSUM"))
        ps_s = ctx.enter_context(tc.tile_pool(name="ps_s", bufs=2, space="PSUM"))
        ps_o1 = ctx.enter_context(tc.tile_pool(name="ps_o1", bufs=2, space="PSUM"))
        ps_o2 = ctx.enter_context(tc.tile_pool(name="ps_o2", bufs=2, space="PSUM"))

        # ---- constants ----
        cst = _LnConsts()
        ident_f = const.tile([P, P], F32)
        make_identity(nc, ident_f)
        ident = const.tile([P, P], F32R)
        nc.vector.tensor_copy(ident[:], ident_f[:])
        cst.eps = const.tile([P, 1], F32)
        nc.vector.memset(cst.eps[:], EPS)
        ones_f = const.tile([P, 1], F32)
        nc.vector.memset(ones_f[:], 1.0)
        zeros_f = const.tile([P, 1], F32)
        nc.vector.memset(zeros_f[:], 0.0)
        cst.lnscale = const.tile([P, 1], F32)
        nc.vector.memset(cst.lnscale[:], float(math.log(SCALE)))
        onezero_r = const.tile([P, 2], F32R)
        nc.vector.tensor_copy(onezero_r[:, 0:1], ones_f[:])
        nc.vector.tensor_copy(onezero_r[:, 1:2], zeros_f[:])
        maskb = const.tile([P, NS], F32)
        nc.sync.dma_start(out=maskb[:], in_=maskb_d[:])
        affine_t = affine_s = None
        if apply_affine:
            wt = const.tile([P, E], F32)
            bt = const.tile([P, E], F32)
            ws = const.tile([P, E], F32)
            bs = const.tile([P, E], F32)
            nc.sync.dma_start(out=wt[:], in_=lnw_t_d[:].partition_broadcast(P))
            nc.sync.dma_start(out=bt[:], in_=lnb_t_d[:].partition_broadcast(P))
            nc.sync.dma_start(out=ws[:], in_=lnw_s_d[:].partition_broadcast(P))
            nc.sync.dma_start(out=bs[:], in_=lnb_s_d[:].partition_broadcast(P))
            affine_t, affine_s = (wt, bt), (ws, bs)

        dma_engines = [nc.sync, nc.scalar]  # SP-HWDGE and ACT-HWDGE queue sets


        # ---- PE warmup: ~3.5us of dummy matmuls from t~0 so the HAM clock
        # gate reaches 8/8 before the first real transpose/matmul ----
        ps_w = ps_tr.tile([P, P], F32, tag="ps_tr", name="ps_warm")
        for w in range(3):
            nc.tensor.matmul(ps_w[:], ident_f[:], ident_f[:], start=True, stop=True)
        warm_sink = const.tile([P, 1], F32)
        nc.vector.tensor_copy(warm_sink[:], ps_w[:, 0:1])

        # ---- LN target (loads on SP queue; 1/sqrt(E) folded into q's rstd) ----
        q = []
        for i in range(NT):
            t_ = q_pool.tile([P, E], F32R, tag=f"q{i}", name=f"q{i}")
            _emit_ln(
                nc, io_pool, stats_pool, cst, target_d, i * P, t_[:],
                nc.sync, affine_t,
                exp_bias=(0.0 if affine_t is not None else cst.lnscale[:]),
            )
            q.append(t_)

        # ---- q transposes -> qT[ec] = [e-chunk 128, T] ----
        qT = [tr_pool.tile([P, T], F32R, name=f"qT{ec}", tag=f"qT{ec}") for ec in range(NE)]
        for g in range(NT // 4):
            for ec in range(NE):
                esl = slice(ec * P, (ec + 1) * P)
                ps = ps_tr.tile([P, 512], F32R, tag="ps_tr", name=f"ps_q{ec}_{g}")
                for tt in range(4):
                    nc.tensor.transpose(
                        ps[:, tt * P : (tt + 1) * P], q[g * 4 + tt][:, esl], ident[:]
                    )
                nc.scalar.copy(out=qT[ec][:, g * 512 : (g + 1) * 512], in_=ps[:])

        # ---- LN source (loads on the ACT HWDGE queue, parallel to q's SP
        # loads).  q is exactly zero-mean over e, so source-side LN commutes
        # past QK^T (mean term multiplies sum_e q = 0):
        # scoresT = r_s * (rawKV^T @ q_scaled).  rscale[:, j] holds r_s. ----
        rscale = tr_pool.tile([P, NS], F32, name="rscale")
        kv = []
        kv_raw = []
        for j in range(NS):
            t_ = kv_pool.tile([P, E + 2], F32R, tag=f"kv{j}", name=f"kv{j}")
            x = _emit_ln(
                nc, io_pool, stats_pool, cst, source_d, j * P, t_[:, 0:E],
                nc.scalar, affine_s,
                rstd_out=(None if affine_s is not None else rscale[:, j : j + 1]),
            )
            nc.vector.tensor_copy(t_[:, E : E + 2], onezero_r[:])
            kv.append(t_)
            kv_raw.append(x)

        kvT = [tr_pool.tile([P, 512], F32R, name=f"kvT{j}", tag=f"kvT{j}") for j in range(NS)]

        # ---- unified j-stream: kv transpose/evict, scoresT+exp for BOTH halves,
        # AV chains for (h0,tt0),(h0,tt1) -- then back-half AV for the rest ----
        NO1 = 256           # AV split: [0:256) and [256:514) incl. denom col (even N for fp32r)
        NO2 = E + 2 - NO1   # 258
        pT = {0: [], 1: []}
        po1 = {}
        po2 = {}
        for (h, tt) in ((0, 0), (0, 1)):
            po1[(h, tt)] = ps_o1.tile([P, NO1], F32, tag="ps_o1", name=f"po1_{h}_{tt}")
            po2[(h, tt)] = ps_o2.tile([P, NO2], F32, tag="ps_o2", name=f"po2_{h}_{tt}")
        for j in range(NS):
            if apply_affine:
                ps = ps_tr.tile([P, 512], F32R, tag="ps_tr", name=f"ps_kv{j}")
                tsrc, tid = kv[j], ident
            else:
                # transpose the RAW source tile: ready straight off the DMA,
                # decoupled from the LN chain (fp32 transpose, 2 cyc/row)
                ps = ps_tr.tile([P, 512], F32, tag="ps_tr", name=f"ps_kv{j}")
                tsrc, tid = kv_raw[j], ident_f
            for ec in range(NE):
                esl = slice(ec * P, (ec + 1) * P)
                nc.tensor.transpose(
                    ps[:, ec * P : (ec + 1) * P], tsrc[:, esl], tid[:]
                )
            nc.vector.tensor_copy(kvT[j][:, 0:256], ps[:, 0:256])
            nc.scalar.copy(out=kvT[j][:, 256:512], in_=ps[:, 256:512])
            for h in range(2):
                tsl = slice(h * 512, (h + 1) * 512)
                ps_sc = ps_s.tile([P, 512], F32, tag="ps_s", name=f"ps_s{h}_{j}")
                for ec in range(NE):
                    nc.tensor.matmul(
                        ps_sc[:],
                        kvT[j][:, ec * P : (ec + 1) * P],
                        qT[ec][:, tsl],
                        start=(ec == 0),
                        stop=(ec == NE - 1),
                    )
                pt = p_pool.tile([P, 512], F32R, tag=f"pT{h}_{j}", name=f"pT{h}_{j}")
                nc.scalar.activation(
                    out=pt[:],
                    in_=ps_sc[:],
                    func=AF.Exp,
                    bias=maskb[:, j : j + 1],
                    scale=(SCALE if apply_affine else rscale[:, j : j + 1]),
                )
                pT[h].append(pt)
            # AV accumulation for (h0,tt0),(h0,tt1) streams alongside
            for (h, tt) in ((0, 0), (0, 1)):
                lhsT = pT[h][j][:, tt * P : (tt + 1) * P]
                nc.tensor.matmul(
                    po1[(h, tt)][:], lhsT, kv[j][:, 0:NO1],
                    start=(j == 0), stop=(j == NS - 1),
                )
                nc.tensor.matmul(
                    po2[(h, tt)][:], lhsT, kv[j][:, NO1 : E + 2],
                    start=(j == 0), stop=(j == NS - 1),
                )

        def _finish_tt(h, tt):
            recip = stats_pool.tile([P, 1], F32, tag="recip", name=f"recip{h}_{tt}")
            nc.vector.reciprocal(out=recip[:], in_=po2[(h, tt)][:, E - NO1 : E - NO1 + 1])
            ot = out_pool.tile([P, E], F32, tag="out", name=f"out{h}_{tt}")
            nc.vector.tensor_scalar_mul(out=ot[:, 0:NO1], in0=po1[(h, tt)][:], scalar1=recip[:])
            nc.scalar.mul(
                out=ot[:, NO1:E], in_=po2[(h, tt)][:, 0 : E - NO1], mul=recip[:]
            )
            row0 = (h * 4 + tt) * P
            nc.sync.dma_start(out=out_d[row0 : row0 + P, :], in_=ot[:])

        _finish_tt(0, 0)
        _finish_tt(0, 1)
        for (h, tt) in ((0, 2), (0, 3), (1, 0), (1, 1), (1, 2), (1, 3)):
            po1[(h, tt)] = ps_o1.tile([P, NO1], F32, tag="ps_o1", name=f"po1_{h}_{tt}")
            po2[(h, tt)] = ps_o2.tile([P, NO2], F32, tag="ps_o2", name=f"po2_{h}_{tt}")
            for j in range(NS):
                lhsT = pT[h][j][:, tt * P : (tt + 1) * P]
                nc.tensor.matmul(
                    po1[(h, tt)][:], lhsT, kv[j][:, 0:NO1],
                    start=(j == 0), stop=(j == NS - 1),
                )
                nc.tensor.matmul(
                    po2[(h, tt)][:], lhsT, kv[j][:, NO1 : E + 2],
                    start=(j == 0), stop=(j == NS - 1),
                )
            _finish_tt(h, tt)

    # Force the act-table-load pass to satisfy Ln+Exp(+Copy) with the single
    # combined `natural_log_exp_and_others` set: hide Exp/Ln from every other
    # set in the dict it sees (positions preserved, so the emitted
    # act_func_set_id still indexes the real act_info.json entry, whose actual
    # contents are a superset of what we use).
    import concourse.bacc as _bacc_mod
    import concourse.hw_specs as _hw_specs

    _orig_tables = _hw_specs.get_activation_tables

    def _patched_tables(arch):
        tabs = {k: set(v) for k, v in _orig_tables(arch).items()}
        for name, fns in tabs.items():
            if name != "natural_log_exp_and_others":
                fns.discard(mybir.ActivationFunctionType.Exp)
                fns.discard(mybir.ActivationFunctionType.Ln)
        return tabs

    _bacc_mod.get_activation_tables = _patched_tables
    try:
        nc.compile()
    finally:
        _bacc_mod.get_activation_tables = _orig_tables
    n_loads = sum(
        1
        for bb in nc.m.functions[0].blocks
        for inst in bb.instructions
        if type(inst).__name__ == "InstLoadActFuncSet"
    )
    assert n_loads <= 2, f"ACT table thrash: {n_loads} loads"
    return nc


def _prep_in_maps(target, source, source_data_mask, apply_affine, lns=None):
    target = np.ascontiguousarray(np.asarray(target, dtype=np.float32))
    source = np.ascontiguousarray(np.asarray(source, dtype=np.float32))
    mask = np.asarray(source_data_mask).astype(bool)
    bias = np.where(mask, 0.0, MASK_NEG).astype(np.float32)  # (N, S)
    in_maps = []
    for i in range(N_CORES):
        m = {
            "target_t": target[i],
            "source_t": source[i],
            "maskbias": np.ascontiguousarray(bias[i].reshape(NS, P).T),
        }
        if apply_affine:
            lnw_t, lnb_t, lnw_s, lnb_s = lns
            m.update(
                lnw_t=np.asarray(lnw_t, np.float32),
                lnb_t=np.asarray(lnb_t, np.float32),
                lnw_s=np.asarray(lnw_s, np.float32),
                lnb_s=np.asarray(lnb_s, np.float32),
            )
        in_maps.append(m)
    return in_maps


def run(target, source, ln_t_w, ln_t_b, ln_s_w, ln_s_b, source_data_mask, **rk):
    """Build (cached), run on 8 cores, return (output, BassKernelResults)."""
    apply_affine = not (
        np.all(np.asarray(ln_t_w) == 1.0)
        and np.all(np.asarray(ln_t_b) == 0.0)
        and np.all(np.asarray(ln_s_w) == 1.0)
        and np.all(np.asarray(ln_s_b) == 0.0)
    )
    if apply_affine not in _cache:
        _cache[apply_affine] = _build(apply_affine)
    nc = _cache[apply_affine]
    in_maps = _prep_in_maps(
        target, source, source_data_mask, apply_affine,
        (ln_t_w, ln_t_b, ln_s_w, ln_s_b),
    )
    res = run_bass_kernel_spmd(nc, in_maps, core_ids=list(range(N_CORES)), **rk)
    out = np.stack([res.results[i]["out_t"] for i in range(N_CORES)], axis=0)
    return out.astype(np.float32), res


def kernel(**inputs) -> np.ndarray:
    out, _ = run(**inputs)
    return out
